# revision 22
# baseline (speedup 1.0000x reference)
"""Trainium2 Bass kernel for nn_CaT_13941463842986 (sparse_attention).

Math (head_size==1 collapses attention to a prefix softmax over T):
  qk[b,h,j]   = c[l,h] * x[b,j]^2            with c = wk*wq
  head_out    = (excl-prefix-sum of E*v) / (excl-prefix-sum of E),
  E = exp(qk), v = x*wv.  Exclusive prefix sums over T=128 are matmuls
against strict-upper-triangular (in [j,i] indexing) ones matrices on the
tensor engine.  |qk| <= ~49 for this data, so exp() needs no max-shift.

Sharding: pure data parallel over batch B=512 -> 64 rows per core x 8 cores.
On-chip layout is T-major: tiles are [T=128 partitions, (b,h) free],
free index = b*8 + h (h innermost).

Key techniques:
 - per-head broadcasts (x, x^2, per-head consts) are stride-0 access
   patterns directly on DVE/Pool compute ops -- no broadcast DMAs
 - 1/den comes from the ACT LUTs: r = exp(-ln(den)); both tri matrices
   carry a 2^-33 scale so ln's input stays inside the LUT's ~2^+-64
   window (num and den scale together, so num'*r' == num/den)
 - the head sum is one strided X-axis tensor_reduce over [T, 64, 8]
 - tri_den[0,0]=1 keeps den>0 on row 0; tri_num keeps the 0 so no
   row-0 fixup is needed
 - all input-derived scalars ride tiles/APs, so the built program is
   input-independent
"""

import numpy as np

import concourse.bass as bass
import concourse.mybir as mybir
from concourse import tile
from concourse.alu_op_type import AluOpType
from concourse.bass_utils import run_bass_kernel_spmd

B, T, H, L = 512, 128, 8, 3
NCORES = 8
BC = B // NCORES  # 64 batch rows per core
W = H * BC  # 512 free width of the (b,h) tiles
HW2 = W // 2
F32 = mybir.dt.float32
F32R = mybir.dt.float32r
AF = mybir.ActivationFunctionType

# ffc const-tile column layout (per layer l at FF0 + l*FBLK):
#   0:4   w1   (W1[l,0,k])
#   4:8   b1'  (W1[l,0,k]*bp[l] + b1[l,k])
#   8:12  w2'  (W2[l,k,0], *w_lm for l==2)
#   12    ybb scale   (1.0, w_lm for l==2)
#   13    ybb bias    (bp+b2, *w_lm + b_lm for l==2)
FBLK = 16
# smalls tensor: [T, SC] = xt(64) | cb_l(8) wb_l(8) x3 | ff(48)
XT0, CB0 = 0, 64
FF0 = CB0 + L * 16
SC = FF0 + L * FBLK

LAST_RESULT = None
_BUILT = None

CSL = [slice(0, HW2), slice(HW2, W)]  # wide column chunks (b 0:32 | 32:64)


def _bcast_bh(xcol, bsl=None):
    """[T,64] tile -> [T,nb,8] stride-0 view (replicate along h)."""
    v = xcol if bsl is None else xcol[:, bsl]
    nb = v.shape[1]
    return v.unsqueeze(2).broadcast_to([T, nb, 8])


def _bcast_h(hrow, nb):
    """[T,8] tile -> [T,nb,8] stride-0 view (replicate along b)."""
    return hrow[:, :].unsqueeze(1).broadcast_to([T, nb, 8])


def _w3(tile_, csl):
    """[T,W] tile chunk -> [T,nb,8] view."""
    return tile_[:, csl].rearrange("p (b h) -> p b h", h=H)


def _build():
    nc = bass.Bass("TRN2", target_bir_lowering=False, debug=False)

    trid_d = nc.dram_tensor("trid", [T, T], F32R, kind="ExternalInput")
    trin_d = nc.dram_tensor("trin", [T, T], F32R, kind="ExternalInput")
    sm_d = nc.dram_tensor("smalls", [T, SC], F32, kind="ExternalInput")
    out_d = nc.dram_tensor("out_t", [T, BC], F32, kind="ExternalOutput")

    with tile.TileContext(nc) as tc:
        with tc.tile_pool(name="const", bufs=1) as cp, tc.tile_pool(
            name="work", bufs=3
        ) as wp, tc.tile_pool(name="psum", bufs=2, space="PSUM") as pp:
            trid = cp.tile([T, T], F32R, tag="trid")
            trin = cp.tile([T, T], F32R, tag="trin")
            sm = cp.tile([T, SC], F32, tag="sm")

            # trigger the ACT table load right away (input values are
            # irrelevant -- this is only a warmup for the LUT load)
            scratch = cp.tile([T, 1], F32, tag="scratch")
            nc.scalar.activation(
                out=scratch[:, :], in_=scratch[:, :], func=AF.Exp
            )

            # loads ride both HW-DGE queues, ordered by first use:
            # sm (x + consts) -> trid (den matmul) -> trin
            SH = SC // 2
            nc.sync.dma_start(out=sm[:, :SH], in_=sm_d[:, :SH])
            nc.scalar.dma_start(out=trid[:, T // 2 :], in_=trid_d[:, T // 2 :])
            nc.sync.dma_start(out=trid[:, : T // 2], in_=trid_d[:, : T // 2])
            nc.scalar.dma_start(out=sm[:, SH:], in_=sm_d[:, SH:])
            nc.sync.dma_start(out=trin[:, : T // 2], in_=trin_d[:, : T // 2])
            nc.scalar.dma_start(out=trin[:, T // 2 :], in_=trin_d[:, T // 2 :])

            xcur = sm[:, XT0 : XT0 + BC]
            for l in range(L):
                fb = FF0 + l * FBLK
                cb = sm[:, CB0 + l * 16 : CB0 + l * 16 + 8]
                wb = sm[:, CB0 + l * 16 + 8 : CB0 + l * 16 + 16]
                u = wp.tile([T, BC], F32, tag="u")
                nc.vector.tensor_tensor(
                    out=u[:, :], in0=xcur[:, :], in1=xcur[:, :],
                    op=AluOpType.mult,
                )
                # qk in two chunks so exp_a can start after qk_a; no
                # other DVE op is ready before qk (ev depends on ee),
                # so the greedy scheduler cannot delay the chain
                qk = wp.tile([T, W], F32, tag="qk")
                for ci in (0, 1):
                    bsl = slice(ci * 32, (ci + 1) * 32)
                    nc.vector.tensor_tensor(
                        out=_w3(qk, CSL[ci]),
                        in0=_bcast_bh(u, bsl),
                        in1=_bcast_h(cb, 32),
                        op=AluOpType.mult,
                    )

                # per-chunk tiles: dependency tracking is tile-granular,
                # so separate tiles let each consumer start as soon as its
                # own chunk's producer is done
                ee = [wp.tile([T, HW2], F32R, tag=f"ee{c}", name=f"ee{c}") for c in (0, 1)]
                ev = [wp.tile([T, HW2], F32R, tag=f"ev{c}", name=f"ev{c}") for c in (0, 1)]
                den = [pp.tile([T, HW2], F32, tag=f"den{c}", name=f"den{c}") for c in (0, 1)]
                num = [pp.tile([T, HW2], F32, tag=f"num{c}", name=f"num{c}") for c in (0, 1)]
                # xw = x*wvp on the otherwise-idle Pool engine.  The tiny
                # dummy op reads qk purely to gate xw behind it in Pool's
                # queue: Pool and DVE share SBUF ports, so an ungated xw
                # would contend with the critical u->qk ops.
                gate = wp.tile([T, 1], F32, tag="gate")
                nc.gpsimd.tensor_tensor(
                    out=gate[:, :], in0=qk[:, W - 1 : W], in1=qk[:, W - 1 : W],
                    op=AluOpType.mult,
                )
                xw = wp.tile([T, W], F32, tag="xw")
                nc.gpsimd.tensor_tensor(
                    out=_w3(xw, slice(0, W)),
                    in0=_bcast_bh(xcur),
                    in1=_bcast_h(wb, BC),
                    op=AluOpType.mult,
                )
                for ci in (0, 1):
                    csl = CSL[ci]
                    nc.scalar.activation(
                        out=ee[ci][:, :], in_=qk[:, csl], func=AF.Exp
                    )
                    nc.vector.tensor_tensor(
                        out=ev[ci][:, :], in0=ee[ci][:, :], in1=xw[:, csl],
                        op=AluOpType.mult,
                    )
                # PE order: both den (tri_den loaded once), then both num
                for ci in (0, 1):
                    nc.tensor.matmul(
                        den[ci][:, :], trid[:, :], ee[ci][:, :],
                        start=True, stop=True,
                    )
                for ci in (0, 1):
                    nc.tensor.matmul(
                        num[ci][:, :], trin[:, :], ev[ci][:, :],
                        start=True, stop=True,
                    )

                # r = 1/den = exp(-ln(den)); ho = num * r, chunk-pipelined
                # against ACT
                ho = wp.tile([T, W], F32, tag="ho")
                for ci in (0, 1):
                    csl = CSL[ci]
                    ld = wp.tile([T, HW2], F32, tag=f"ld{ci}", name=f"ld{ci}")
                    r = wp.tile([T, HW2], F32, tag=f"r{ci}", name=f"r{ci}")
                    nc.scalar.activation(
                        out=ld[:, :], in_=den[ci][:, :], func=AF.Ln
                    )
                    nc.scalar.activation(
                        out=r[:, :], in_=ld[:, :], func=AF.Exp, scale=-1.0
                    )
                    nc.vector.tensor_tensor(
                        out=ho[:, csl], in0=num[ci][:, :], in1=r[:, :],
                        op=AluOpType.mult,
                    )
                y0 = wp.tile([T, BC], F32, tag="y0")
                nc.vector.tensor_reduce(
                    out=y0[:, :],
                    in_=ho[:, :].rearrange("p (b h) -> p b h", h=H),
                    axis=mybir.AxisListType.X,
                    op=AluOpType.add,
                )

                # FF: xn = ybb + sum_k w2'_k * relu(w1_k*y0 + b1'_k)
                ybb = wp.tile([T, BC], F32, tag="ybb")
                nc.vector.tensor_scalar(
                    out=ybb[:, :], in0=y0[:, :],
                    scalar1=sm[:, fb + 12 : fb + 13],
                    scalar2=sm[:, fb + 13 : fb + 14],
                    op0=AluOpType.mult,
                    op1=AluOpType.add,
                )
                rk = wp.tile([T, 4 * BC], F32, tag="rk")
                for k in range(4):
                    nc.scalar.activation(
                        out=rk[:, k * BC : (k + 1) * BC], in_=y0[:, :],
                        func=AF.Relu,
                        scale=sm[:, fb + k : fb + k + 1],
                        bias=sm[:, fb + 4 + k : fb + 5 + k],
                    )
                q = ybb
                for k in range(4):
                    qn = wp.tile([T, BC], F32, tag=f"q{k}", name=f"q{k}")
                    nc.vector.scalar_tensor_tensor(
                        out=qn[:, :],
                        in0=rk[:, k * BC : (k + 1) * BC],
                        scalar=sm[:, fb + 8 + k : fb + 9 + k],
                        in1=q[:, :],
                        op0=AluOpType.mult,
                        op1=AluOpType.add,
                    )
                    q = qn
                xcur = q

            nc.sync.dma_start(out=out_d[:, :], in_=xcur[:, :])

    return nc


def _split_multi_waits(nc):
    """This container's walrus accepts only one embedded sem wait per
    instruction; hoist extra waits onto same-engine EventSemaphore ops.
    Custom-DVE ISA ops can't carry any embedded sync at all."""
    nid = 0
    for fn in nc.m.functions:
        for blk in fn.blocks:
            insts = blk.instructions
            i = 0
            while i < len(insts):
                ins = insts[i]
                si = getattr(ins, "sync_info", None)
                is_custom = isinstance(ins, mybir.InstCustomDveAnt)
                is_raw_isa = isinstance(ins, mybir.InstISA) and not is_custom
                keep = 0 if is_custom else 1
                if si is not None and len(si.on_wait) > keep and not is_raw_isa:
                    waits = list(si.on_wait)
                    split, kept = (
                        (waits, []) if keep == 0 else (waits[:-1], [waits[-1]])
                    )
                    for w in split:
                        ev = mybir.InstEventSemaphore(
                            name=f"WSPLIT-{nid}", ins=[], outs=[]
                        )
                        nid += 1
                        ev.engine = ins.engine
                        ev.sync_info = mybir.SyncInfo(on_wait=[w], on_update=[])
                        insts.insert(i, ev)
                        i += 1
                    ins.sync_info = mybir.SyncInfo(
                        on_wait=kept, on_update=list(si.on_update)
                    )
                    si = ins.sync_info
                if is_custom and si is not None and len(si.on_update) > 0:
                    ev = mybir.InstEventSemaphore(
                        name=f"WSPLIT-{nid}", ins=[], outs=[]
                    )
                    nid += 1
                    ev.engine = ins.engine
                    ev.sync_info = mybir.SyncInfo(
                        on_wait=[], on_update=list(si.on_update)
                    )
                    ins.sync_info = mybir.SyncInfo(
                        on_wait=list(si.on_wait), on_update=[]
                    )
                    insts.insert(i + 1, ev)
                    i += 1
                i += 1


def _get_built():
    global _BUILT
    if _BUILT is None:
        _BUILT = _build()
        _split_multi_waits(_BUILT)
    return _BUILT


def _host_inputs(X, wk, wq, wv, Wp, bp, W1, b1, W2, b2, w_lm, b_lm):
    c = wk * wq  # [L,H]
    wvp = wv * Wp[:, :, 0]  # [L,H]
    # [j,i] = 1 if j<i; 2^-33 scale keeps ln(den') in the Ln LUT window
    trin = np.triu(np.ones((T, T), np.float32), 1) * 2.0**-33
    trid = trin.copy()
    trid[0, 0] = 2.0**-33  # den row0 = E[0,:] keeps den>0; num row0 stays 0

    XT = np.ascontiguousarray(X.T.astype(np.float32))  # [T, B]

    # smalls (identical across cores): [T, SC]
    sm_common = np.zeros((1, SC), np.float32)
    for l in range(L):
        base = CB0 + l * 16
        sm_common[0, base : base + 8] = c[l]
        sm_common[0, base + 8 : base + 16] = wvp[l]
    for l in range(L):
        fb = FF0 + l * FBLK
        lm_s = float(w_lm[0]) if l == L - 1 else 1.0
        lm_b = float(b_lm[0]) if l == L - 1 else 0.0
        sm_common[0, fb : fb + 4] = W1[l, 0, :]
        sm_common[0, fb + 4 : fb + 8] = W1[l, 0, :] * bp[l, 0] + b1[l]
        sm_common[0, fb + 8 : fb + 12] = W2[l, :, 0] * lm_s
        sm_common[0, fb + 12] = lm_s
        sm_common[0, fb + 13] = (bp[l, 0] + b2[l, 0]) * lm_s + lm_b
    sm = np.ascontiguousarray(np.broadcast_to(sm_common, (T, SC)), np.float32)

    in_maps = []
    for core in range(NCORES):
        smc = sm.copy()
        smc[:, XT0 : XT0 + BC] = XT[:, core * BC : (core + 1) * BC]
        in_maps.append(
            {
                "trid": trid,
                "trin": trin,
                "smalls": np.ascontiguousarray(smc),
            }
        )
    return in_maps


def kernel(X, wk, wq, wv, Wp, bp, W1, b1, W2, b2, w_lm, b_lm):
    global LAST_RESULT
    args = [
        np.asarray(a, np.float32)
        for a in (X, wk, wq, wv, Wp, bp, W1, b1, W2, b2, w_lm, b_lm)
    ]
    nc = _get_built()
    in_maps = _host_inputs(*args)
    res = run_bass_kernel_spmd(nc, in_maps, core_ids=list(range(NCORES)))
    LAST_RESULT = res

    out = np.empty((B, T), np.float32)
    for core in range(NCORES):
        out[core * BC : (core + 1) * BC, :] = res.results[core]["out_t"].T
    return out


# revision 23
# speedup vs baseline: 1.0749x; 1.0749x over previous
"""Trainium2 Bass kernel for nn_CaT_13941463842986 (sparse_attention).

Math (head_size==1 collapses attention to a prefix softmax over T):
  qk[b,h,j]   = c[l,h] * x[b,j]^2            with c = wk*wq
  head_out    = (excl-prefix-sum of E*v) / (excl-prefix-sum of E),
  E = exp(qk), v = x*wv.  Exclusive prefix sums over T=128 are matmuls
against strict-upper-triangular (in [j,i] indexing) ones matrices on the
tensor engine.  |qk| <= ~49 for this data, so exp() needs no max-shift.

Sharding: pure data parallel over batch B=512 -> 64 rows per core x 8 cores.
On-chip layout is T-major: tiles are [T=128 partitions, (b,h) free],
free index = b*8 + h (h innermost).

Key techniques:
 - per-head broadcasts (x, x^2, per-head consts) are stride-0 access
   patterns directly on DVE/Pool compute ops -- no broadcast DMAs
 - 1/den comes from the ACT LUTs: r = exp(-ln(den)); both tri matrices
   carry a 2^-33 scale so ln's input stays inside the LUT's ~2^+-64
   window (num and den scale together, so num'*r' == num/den)
 - the head sum is one strided X-axis tensor_reduce over [T, 64, 8]
 - tri_den[0,0]=1 keeps den>0 on row 0; tri_num keeps the 0 so no
   row-0 fixup is needed
 - all input-derived scalars ride tiles/APs, so the built program is
   input-independent
"""

import numpy as np

import concourse.bass as bass
import concourse.mybir as mybir
from concourse import tile
from concourse.alu_op_type import AluOpType
from concourse.bass_utils import run_bass_kernel_spmd

B, T, H, L = 512, 128, 8, 3
NCORES = 8
BC = B // NCORES  # 64 batch rows per core
W = H * BC  # 512 free width of the (b,h) tiles
HW2 = W // 2
F32 = mybir.dt.float32
F32R = mybir.dt.float32r
AF = mybir.ActivationFunctionType

# ffc const-tile column layout (per layer l at FF0 + l*FBLK):
#   0:4   w1   (W1[l,0,k])
#   4:8   b1'  (W1[l,0,k]*bp[l] + b1[l,k])
#   8:12  w2'  (W2[l,k,0], *w_lm for l==2)
#   12    ybb scale   (1.0, w_lm for l==2)
#   13    ybb bias    (bp+b2, *w_lm + b_lm for l==2)
FBLK = 16
# smalls tensor: [T, SC] = xt(64) | cb_l(8) wb_l(8) x3 | ff(48)
XT0, CB0 = 0, 64
FF0 = CB0 + L * 16
SC = FF0 + L * FBLK

LAST_RESULT = None
_BUILT = None

CSL = [slice(0, HW2), slice(HW2, W)]  # wide column chunks (b 0:32 | 32:64)


def _bcast_bh(xcol, bsl=None):
    """[T,64] tile -> [T,nb,8] stride-0 view (replicate along h)."""
    v = xcol if bsl is None else xcol[:, bsl]
    nb = v.shape[1]
    return v.unsqueeze(2).broadcast_to([T, nb, 8])


def _bcast_h(hrow, nb):
    """[T,8] tile -> [T,nb,8] stride-0 view (replicate along b)."""
    return hrow[:, :].unsqueeze(1).broadcast_to([T, nb, 8])


def _w3(tile_, csl):
    """[T,W] tile chunk -> [T,nb,8] view."""
    return tile_[:, csl].rearrange("p (b h) -> p b h", h=H)


def _build():
    nc = bass.Bass("TRN2", target_bir_lowering=False, debug=False)

    trid_d = nc.dram_tensor("trid", [T, T], F32R, kind="ExternalInput")
    trin_d = nc.dram_tensor("trin", [T, T], F32R, kind="ExternalInput")
    sm_d = nc.dram_tensor("smalls", [T, SC], F32, kind="ExternalInput")
    out_d = nc.dram_tensor("out_t", [T, BC], F32, kind="ExternalOutput")

    with tile.TileContext(nc) as tc:
        with tc.tile_pool(name="const", bufs=1) as cp, tc.tile_pool(
            name="work", bufs=3
        ) as wp, tc.tile_pool(name="psum", bufs=2, space="PSUM") as pp:
            trid = cp.tile([T, T], F32R, tag="trid")
            trin = cp.tile([T, T], F32R, tag="trin")
            sm = cp.tile([T, SC], F32, tag="sm")

            # trigger the ACT table load right away (input values are
            # irrelevant -- this is only a warmup for the LUT load)
            scratch = cp.tile([T, 1], F32, tag="scratch")
            nc.scalar.activation(
                out=scratch[:, :], in_=scratch[:, :], func=AF.Exp
            )

            # loads ride both HW-DGE queues, ordered by first use:
            # sm (x + consts) -> trid (den matmul) -> trin
            SH = SC // 2
            nc.sync.dma_start(out=sm[:, :SH], in_=sm_d[:, :SH])
            nc.scalar.dma_start(out=trid[:, T // 2 :], in_=trid_d[:, T // 2 :])
            nc.sync.dma_start(out=trid[:, : T // 2], in_=trid_d[:, : T // 2])
            nc.scalar.dma_start(out=sm[:, SH:], in_=sm_d[:, SH:])
            nc.sync.dma_start(out=trin[:, : T // 2], in_=trin_d[:, : T // 2])
            nc.scalar.dma_start(out=trin[:, T // 2 :], in_=trin_d[:, T // 2 :])

            xcur = sm[:, XT0 : XT0 + BC]
            for l in range(L):
                fb = FF0 + l * FBLK
                cb = sm[:, CB0 + l * 16 : CB0 + l * 16 + 8]
                wb = sm[:, CB0 + l * 16 + 8 : CB0 + l * 16 + 16]
                u = wp.tile([T, BC], F32, tag="u")
                nc.vector.tensor_tensor(
                    out=u[:, :], in0=xcur[:, :], in1=xcur[:, :],
                    op=AluOpType.mult,
                )
                # qk in two chunks so exp_a can start after qk_a; no
                # other DVE op is ready before qk (ev depends on ee),
                # so the greedy scheduler cannot delay the chain
                qk = wp.tile([T, W], F32, tag="qk")
                for ci in (0, 1):
                    bsl = slice(ci * 32, (ci + 1) * 32)
                    nc.vector.tensor_tensor(
                        out=_w3(qk, CSL[ci]),
                        in0=_bcast_bh(u, bsl),
                        in1=_bcast_h(cb, 32),
                        op=AluOpType.mult,
                    )

                # per-chunk tiles: dependency tracking is tile-granular,
                # so separate tiles let each consumer start as soon as its
                # own chunk's producer is done
                ee = [wp.tile([T, HW2], F32R, tag=f"ee{c}", name=f"ee{c}") for c in (0, 1)]
                ev = [wp.tile([T, HW2], F32R, tag=f"ev{c}", name=f"ev{c}") for c in (0, 1)]
                den = [pp.tile([T, HW2], F32, tag=f"den{c}", name=f"den{c}") for c in (0, 1)]
                num = [pp.tile([T, HW2], F32, tag=f"num{c}", name=f"num{c}") for c in (0, 1)]
                for ci in (0, 1):
                    csl = CSL[ci]
                    bsl = slice(ci * 32, (ci + 1) * 32)
                    nc.scalar.activation(
                        out=ee[ci][:, :], in_=qk[:, csl], func=AF.Exp
                    )
                    # ev = (ee * x_bcast) * wvp_bcast; making both
                    # multiplies depend on ee keeps the DVE ready-queue
                    # empty until qk is done
                    e2 = wp.tile([T, HW2], F32, tag=f"e2{ci}", name=f"e2{ci}")
                    nc.vector.tensor_tensor(
                        out=_w3(e2, slice(0, HW2)),
                        in0=ee[ci][:, :].rearrange("p (b h) -> p b h", h=H),
                        in1=_bcast_bh(xcur, bsl),
                        op=AluOpType.mult,
                    )
                    nc.vector.tensor_tensor(
                        out=_w3(ev[ci], slice(0, HW2)),
                        in0=_w3(e2, slice(0, HW2)),
                        in1=_bcast_h(wb, 32),
                        op=AluOpType.mult,
                    )
                # PE order: both den (tri_den loaded once), then both num
                for ci in (0, 1):
                    nc.tensor.matmul(
                        den[ci][:, :], trid[:, :], ee[ci][:, :],
                        start=True, stop=True,
                    )
                for ci in (0, 1):
                    nc.tensor.matmul(
                        num[ci][:, :], trin[:, :], ev[ci][:, :],
                        start=True, stop=True,
                    )

                # r = 1/den = exp(-ln(den)); ho = num * r, chunk-pipelined
                # against ACT
                ho = wp.tile([T, W], F32, tag="ho")
                for ci in (0, 1):
                    csl = CSL[ci]
                    ld = wp.tile([T, HW2], F32, tag=f"ld{ci}", name=f"ld{ci}")
                    r = wp.tile([T, HW2], F32, tag=f"r{ci}", name=f"r{ci}")
                    nc.scalar.activation(
                        out=ld[:, :], in_=den[ci][:, :], func=AF.Ln
                    )
                    nc.scalar.activation(
                        out=r[:, :], in_=ld[:, :], func=AF.Exp, scale=-1.0
                    )
                    nc.vector.tensor_tensor(
                        out=ho[:, csl], in0=num[ci][:, :], in1=r[:, :],
                        op=AluOpType.mult,
                    )
                y0 = wp.tile([T, BC], F32, tag="y0")
                nc.vector.tensor_reduce(
                    out=y0[:, :],
                    in_=ho[:, :].rearrange("p (b h) -> p b h", h=H),
                    axis=mybir.AxisListType.X,
                    op=AluOpType.add,
                )

                # FF: xn = ybb + sum_k w2'_k * relu(w1_k*y0 + b1'_k)
                ybb = wp.tile([T, BC], F32, tag="ybb")
                nc.vector.tensor_scalar(
                    out=ybb[:, :], in0=y0[:, :],
                    scalar1=sm[:, fb + 12 : fb + 13],
                    scalar2=sm[:, fb + 13 : fb + 14],
                    op0=AluOpType.mult,
                    op1=AluOpType.add,
                )
                rk = wp.tile([T, 4 * BC], F32, tag="rk")
                for k in range(4):
                    nc.scalar.activation(
                        out=rk[:, k * BC : (k + 1) * BC], in_=y0[:, :],
                        func=AF.Relu,
                        scale=sm[:, fb + k : fb + k + 1],
                        bias=sm[:, fb + 4 + k : fb + 5 + k],
                    )
                q = ybb
                for k in range(4):
                    qn = wp.tile([T, BC], F32, tag=f"q{k}", name=f"q{k}")
                    nc.vector.scalar_tensor_tensor(
                        out=qn[:, :],
                        in0=rk[:, k * BC : (k + 1) * BC],
                        scalar=sm[:, fb + 8 + k : fb + 9 + k],
                        in1=q[:, :],
                        op0=AluOpType.mult,
                        op1=AluOpType.add,
                    )
                    q = qn
                xcur = q

            nc.sync.dma_start(out=out_d[:, :], in_=xcur[:, :])

    return nc


def _split_multi_waits(nc):
    """This container's walrus accepts only one embedded sem wait per
    instruction; hoist extra waits onto same-engine EventSemaphore ops.
    Custom-DVE ISA ops can't carry any embedded sync at all."""
    nid = 0
    for fn in nc.m.functions:
        for blk in fn.blocks:
            insts = blk.instructions
            i = 0
            while i < len(insts):
                ins = insts[i]
                si = getattr(ins, "sync_info", None)
                is_custom = isinstance(ins, mybir.InstCustomDveAnt)
                is_raw_isa = isinstance(ins, mybir.InstISA) and not is_custom
                keep = 0 if is_custom else 1
                if si is not None and len(si.on_wait) > keep and not is_raw_isa:
                    waits = list(si.on_wait)
                    split, kept = (
                        (waits, []) if keep == 0 else (waits[:-1], [waits[-1]])
                    )
                    for w in split:
                        ev = mybir.InstEventSemaphore(
                            name=f"WSPLIT-{nid}", ins=[], outs=[]
                        )
                        nid += 1
                        ev.engine = ins.engine
                        ev.sync_info = mybir.SyncInfo(on_wait=[w], on_update=[])
                        insts.insert(i, ev)
                        i += 1
                    ins.sync_info = mybir.SyncInfo(
                        on_wait=kept, on_update=list(si.on_update)
                    )
                    si = ins.sync_info
                if is_custom and si is not None and len(si.on_update) > 0:
                    ev = mybir.InstEventSemaphore(
                        name=f"WSPLIT-{nid}", ins=[], outs=[]
                    )
                    nid += 1
                    ev.engine = ins.engine
                    ev.sync_info = mybir.SyncInfo(
                        on_wait=[], on_update=list(si.on_update)
                    )
                    ins.sync_info = mybir.SyncInfo(
                        on_wait=list(si.on_wait), on_update=[]
                    )
                    insts.insert(i + 1, ev)
                    i += 1
                i += 1


def _get_built():
    global _BUILT
    if _BUILT is None:
        _BUILT = _build()
        _split_multi_waits(_BUILT)
    return _BUILT


def _host_inputs(X, wk, wq, wv, Wp, bp, W1, b1, W2, b2, w_lm, b_lm):
    c = wk * wq  # [L,H]
    wvp = wv * Wp[:, :, 0]  # [L,H]
    # [j,i] = 1 if j<i; 2^-33 scale keeps ln(den') in the Ln LUT window
    trin = np.triu(np.ones((T, T), np.float32), 1) * 2.0**-33
    trid = trin.copy()
    trid[0, 0] = 2.0**-33  # den row0 = E[0,:] keeps den>0; num row0 stays 0

    XT = np.ascontiguousarray(X.T.astype(np.float32))  # [T, B]

    # smalls (identical across cores): [T, SC]
    sm_common = np.zeros((1, SC), np.float32)
    for l in range(L):
        base = CB0 + l * 16
        sm_common[0, base : base + 8] = c[l]
        sm_common[0, base + 8 : base + 16] = wvp[l]
    for l in range(L):
        fb = FF0 + l * FBLK
        lm_s = float(w_lm[0]) if l == L - 1 else 1.0
        lm_b = float(b_lm[0]) if l == L - 1 else 0.0
        sm_common[0, fb : fb + 4] = W1[l, 0, :]
        sm_common[0, fb + 4 : fb + 8] = W1[l, 0, :] * bp[l, 0] + b1[l]
        sm_common[0, fb + 8 : fb + 12] = W2[l, :, 0] * lm_s
        sm_common[0, fb + 12] = lm_s
        sm_common[0, fb + 13] = (bp[l, 0] + b2[l, 0]) * lm_s + lm_b
    sm = np.ascontiguousarray(np.broadcast_to(sm_common, (T, SC)), np.float32)

    in_maps = []
    for core in range(NCORES):
        smc = sm.copy()
        smc[:, XT0 : XT0 + BC] = XT[:, core * BC : (core + 1) * BC]
        in_maps.append(
            {
                "trid": trid,
                "trin": trin,
                "smalls": np.ascontiguousarray(smc),
            }
        )
    return in_maps


def kernel(X, wk, wq, wv, Wp, bp, W1, b1, W2, b2, w_lm, b_lm):
    global LAST_RESULT
    args = [
        np.asarray(a, np.float32)
        for a in (X, wk, wq, wv, Wp, bp, W1, b1, W2, b2, w_lm, b_lm)
    ]
    nc = _get_built()
    in_maps = _host_inputs(*args)
    res = run_bass_kernel_spmd(nc, in_maps, core_ids=list(range(NCORES)))
    LAST_RESULT = res

    out = np.empty((B, T), np.float32)
    for core in range(NCORES):
        out[core * BC : (core + 1) * BC, :] = res.results[core]["out_t"].T
    return out


# revision 24
# speedup vs baseline: 1.0853x; 1.0097x over previous
"""Trainium2 Bass kernel for nn_CaT_13941463842986 (sparse_attention).

Math (head_size==1 collapses attention to a prefix softmax over T):
  qk[b,h,j]   = c[l,h] * x[b,j]^2            with c = wk*wq
  head_out    = (excl-prefix-sum of E*v) / (excl-prefix-sum of E),
  E = exp(qk), v = x*wv.  Exclusive prefix sums over T=128 are matmuls
against strict-upper-triangular (in [j,i] indexing) ones matrices on the
tensor engine.  |qk| <= ~49 for this data, so exp() needs no max-shift.

Sharding: pure data parallel over batch B=512 -> 64 rows per core x 8 cores.
On-chip layout is T-major: tiles are [T=128 partitions, (b,h) free],
free index = b*8 + h (h innermost).

Key techniques:
 - per-head broadcasts (x, x^2, per-head consts) are stride-0 access
   patterns directly on DVE compute ops -- no broadcast DMAs at all
 - 1/den comes from the ACT LUTs: r = exp(-ln(den)); both tri matrices
   carry a 2^-33 scale so ln's input stays inside the LUT's ~2^+-64
   window (num and den scale together, so num'*r' == num/den)
 - the head sum is one strided X-axis tensor_reduce over [T, 64, 8]
 - tri_den[0,0]=1 keeps den>0 on row 0; tri_num keeps the 0 so no
   row-0 fixup is needed
 - per-chunk tiles + tuned emission order keep every consumer's
   engine-counter wait on its true producer; ev is computed as
   (ee*x)*wvp so no DVE op is ready before the critical qk, which
   the greedy readiness scheduler would otherwise delay
 - everything elementwise lives on DVE/ACT: GpSimd shares SBUF ports
   with DVE and measurably halves both engines' throughput when run
   concurrently
 - all input-derived scalars ride tiles/APs, so the built program is
   input-independent
"""

import numpy as np

import concourse.bass as bass
import concourse.mybir as mybir
from concourse import tile
from concourse.alu_op_type import AluOpType
from concourse.bass_utils import run_bass_kernel_spmd

B, T, H, L = 512, 128, 8, 3
NCORES = 8
BC = B // NCORES  # 64 batch rows per core
W = H * BC  # 512 free width of the (b,h) tiles
HW2 = W // 2
F32 = mybir.dt.float32
F32R = mybir.dt.float32r
AF = mybir.ActivationFunctionType

# ffc const-tile column layout (per layer l at FF0 + l*FBLK):
#   0:4   w1   (W1[l,0,k])
#   4:8   b1'  (W1[l,0,k]*bp[l] + b1[l,k])
#   8:12  w2'  (W2[l,k,0], *w_lm for l==2)
#   12    ybb scale   (1.0, w_lm for l==2)
#   13    ybb bias    (bp+b2, *w_lm + b_lm for l==2)
FBLK = 16
# smalls tensor: [T, SC] = xt(64) | cb_l(8) wb_l(8) x3 | ff(48)
XT0, CB0 = 0, 64
FF0 = CB0 + L * 16
SC = FF0 + L * FBLK

LAST_RESULT = None
_BUILT = None

CSL = [slice(0, HW2), slice(HW2, W)]  # wide column chunks (b 0:32 | 32:64)


def _bcast_bh(xcol, bsl=None):
    """[T,64] tile -> [T,nb,8] stride-0 view (replicate along h)."""
    v = xcol if bsl is None else xcol[:, bsl]
    nb = v.shape[1]
    return v.unsqueeze(2).broadcast_to([T, nb, 8])


def _bcast_h(hrow, nb):
    """[T,8] tile -> [T,nb,8] stride-0 view (replicate along b)."""
    return hrow[:, :].unsqueeze(1).broadcast_to([T, nb, 8])


def _w3(tile_, csl):
    """[T,W] tile chunk -> [T,nb,8] view."""
    return tile_[:, csl].rearrange("p (b h) -> p b h", h=H)


def _build():
    nc = bass.Bass("TRN2", target_bir_lowering=False, debug=False)

    trid_d = nc.dram_tensor("trid", [T, T], F32R, kind="ExternalInput")
    trin_d = nc.dram_tensor("trin", [T, T], F32R, kind="ExternalInput")
    sm_d = nc.dram_tensor("smalls", [T, SC], F32, kind="ExternalInput")
    out_d = nc.dram_tensor("out_t", [T, BC], F32, kind="ExternalOutput")

    with tile.TileContext(nc) as tc:
        with tc.tile_pool(name="const", bufs=1) as cp, tc.tile_pool(
            name="work", bufs=3
        ) as wp, tc.tile_pool(name="psum", bufs=2, space="PSUM") as pp:
            trid = cp.tile([T, T], F32R, tag="trid")
            trin = cp.tile([T, T], F32R, tag="trin")
            sm = cp.tile([T, SC], F32, tag="sm")

            # trigger the ACT table load right away (input values are
            # irrelevant -- this is only a warmup for the LUT load)
            scratch = cp.tile([T, 1], F32, tag="scratch")
            nc.scalar.activation(
                out=scratch[:, :], in_=scratch[:, :], func=AF.Exp
            )

            # loads ride both HW-DGE queues, ordered by first use:
            # sm (x + consts) -> trid (den matmul) -> trin
            SH = SC // 2
            nc.sync.dma_start(out=sm[:, :SH], in_=sm_d[:, :SH])
            nc.scalar.dma_start(out=trid[:, T // 2 :], in_=trid_d[:, T // 2 :])
            nc.sync.dma_start(out=trid[:, : T // 2], in_=trid_d[:, : T // 2])
            nc.scalar.dma_start(out=sm[:, SH:], in_=sm_d[:, SH:])
            nc.sync.dma_start(out=trin[:, : T // 2], in_=trin_d[:, : T // 2])
            nc.scalar.dma_start(out=trin[:, T // 2 :], in_=trin_d[:, T // 2 :])

            xcur = sm[:, XT0 : XT0 + BC]
            for l in range(L):
                fb = FF0 + l * FBLK
                cb = sm[:, CB0 + l * 16 : CB0 + l * 16 + 8]
                wb = sm[:, CB0 + l * 16 + 8 : CB0 + l * 16 + 16]
                u = wp.tile([T, BC], F32, tag="u")
                nc.vector.tensor_tensor(
                    out=u[:, :], in0=xcur[:, :], in1=xcur[:, :],
                    op=AluOpType.mult,
                )
                # qk in two chunks so exp_a can start after qk_a; no
                # other DVE op is ready before qk (ev depends on ee),
                # so the greedy scheduler cannot delay the chain
                qk = wp.tile([T, W], F32, tag="qk")
                for ci in (0, 1):
                    bsl = slice(ci * 32, (ci + 1) * 32)
                    nc.vector.tensor_tensor(
                        out=_w3(qk, CSL[ci]),
                        in0=_bcast_bh(u, bsl),
                        in1=_bcast_h(cb, 32),
                        op=AluOpType.mult,
                    )

                # per-chunk tiles: dependency tracking is tile-granular,
                # so separate tiles let each consumer start as soon as its
                # own chunk's producer is done
                ee = [wp.tile([T, HW2], F32R, tag=f"ee{c}", name=f"ee{c}") for c in (0, 1)]
                ev = [wp.tile([T, HW2], F32R, tag=f"ev{c}", name=f"ev{c}") for c in (0, 1)]
                den = [pp.tile([T, HW2], F32, tag=f"den{c}", name=f"den{c}") for c in (0, 1)]
                num = [pp.tile([T, HW2], F32, tag=f"num{c}", name=f"num{c}") for c in (0, 1)]
                for ci in (0, 1):
                    csl = CSL[ci]
                    bsl = slice(ci * 32, (ci + 1) * 32)
                    nc.scalar.activation(
                        out=ee[ci][:, :], in_=qk[:, csl], func=AF.Exp
                    )
                    # ev = (ee * x_bcast) * wvp_bcast; making both
                    # multiplies depend on ee keeps the DVE ready-queue
                    # empty until qk is done
                    e2 = wp.tile([T, HW2], F32, tag=f"e2{ci}", name=f"e2{ci}")
                    nc.vector.tensor_tensor(
                        out=_w3(e2, slice(0, HW2)),
                        in0=ee[ci][:, :].rearrange("p (b h) -> p b h", h=H),
                        in1=_bcast_bh(xcur, bsl),
                        op=AluOpType.mult,
                    )
                    nc.vector.tensor_tensor(
                        out=_w3(ev[ci], slice(0, HW2)),
                        in0=_w3(e2, slice(0, HW2)),
                        in1=_bcast_h(wb, 32),
                        op=AluOpType.mult,
                    )
                # PE order: both den (tri_den loaded once), then both num
                for ci in (0, 1):
                    nc.tensor.matmul(
                        den[ci][:, :], trid[:, :], ee[ci][:, :],
                        start=True, stop=True,
                    )
                for ci in (0, 1):
                    nc.tensor.matmul(
                        num[ci][:, :], trin[:, :], ev[ci][:, :],
                        start=True, stop=True,
                    )

                # r = 1/den = exp(-ln(den)); ho = num * r, chunk-pipelined
                # against ACT
                ho = wp.tile([T, W], F32, tag="ho")
                for ci in (0, 1):
                    csl = CSL[ci]
                    ld = wp.tile([T, HW2], F32, tag=f"ld{ci}", name=f"ld{ci}")
                    r = wp.tile([T, HW2], F32, tag=f"r{ci}", name=f"r{ci}")
                    nc.scalar.activation(
                        out=ld[:, :], in_=den[ci][:, :], func=AF.Ln
                    )
                    nc.scalar.activation(
                        out=r[:, :], in_=ld[:, :], func=AF.Exp, scale=-1.0
                    )
                    nc.vector.tensor_tensor(
                        out=ho[:, csl], in0=num[ci][:, :], in1=r[:, :],
                        op=AluOpType.mult,
                    )
                y0 = wp.tile([T, BC], F32, tag="y0")
                nc.vector.tensor_reduce(
                    out=y0[:, :],
                    in_=ho[:, :].rearrange("p (b h) -> p b h", h=H),
                    axis=mybir.AxisListType.X,
                    op=AluOpType.add,
                )

                # FF: xn = ybb + sum_k w2'_k * relu(w1_k*y0 + b1'_k)
                ybb = wp.tile([T, BC], F32, tag="ybb")
                nc.vector.tensor_scalar(
                    out=ybb[:, :], in0=y0[:, :],
                    scalar1=sm[:, fb + 12 : fb + 13],
                    scalar2=sm[:, fb + 13 : fb + 14],
                    op0=AluOpType.mult,
                    op1=AluOpType.add,
                )
                rk = wp.tile([T, 4 * BC], F32, tag="rk")
                for k in range(4):
                    nc.scalar.activation(
                        out=rk[:, k * BC : (k + 1) * BC], in_=y0[:, :],
                        func=AF.Relu,
                        scale=sm[:, fb + k : fb + k + 1],
                        bias=sm[:, fb + 4 + k : fb + 5 + k],
                    )
                q = ybb
                for k in range(4):
                    qn = wp.tile([T, BC], F32, tag=f"q{k}", name=f"q{k}")
                    nc.vector.scalar_tensor_tensor(
                        out=qn[:, :],
                        in0=rk[:, k * BC : (k + 1) * BC],
                        scalar=sm[:, fb + 8 + k : fb + 9 + k],
                        in1=q[:, :],
                        op0=AluOpType.mult,
                        op1=AluOpType.add,
                    )
                    q = qn
                xcur = q

            nc.sync.dma_start(out=out_d[:, :], in_=xcur[:, :])

    return nc


def _split_multi_waits(nc):
    """This container's walrus accepts only one embedded sem wait per
    instruction; hoist extra waits onto same-engine EventSemaphore ops.
    Custom-DVE ISA ops can't carry any embedded sync at all."""
    nid = 0
    for fn in nc.m.functions:
        for blk in fn.blocks:
            insts = blk.instructions
            i = 0
            while i < len(insts):
                ins = insts[i]
                si = getattr(ins, "sync_info", None)
                is_custom = isinstance(ins, mybir.InstCustomDveAnt)
                is_raw_isa = isinstance(ins, mybir.InstISA) and not is_custom
                keep = 0 if is_custom else 1
                if si is not None and len(si.on_wait) > keep and not is_raw_isa:
                    waits = list(si.on_wait)
                    split, kept = (
                        (waits, []) if keep == 0 else (waits[:-1], [waits[-1]])
                    )
                    for w in split:
                        ev = mybir.InstEventSemaphore(
                            name=f"WSPLIT-{nid}", ins=[], outs=[]
                        )
                        nid += 1
                        ev.engine = ins.engine
                        ev.sync_info = mybir.SyncInfo(on_wait=[w], on_update=[])
                        insts.insert(i, ev)
                        i += 1
                    ins.sync_info = mybir.SyncInfo(
                        on_wait=kept, on_update=list(si.on_update)
                    )
                    si = ins.sync_info
                if is_custom and si is not None and len(si.on_update) > 0:
                    ev = mybir.InstEventSemaphore(
                        name=f"WSPLIT-{nid}", ins=[], outs=[]
                    )
                    nid += 1
                    ev.engine = ins.engine
                    ev.sync_info = mybir.SyncInfo(
                        on_wait=[], on_update=list(si.on_update)
                    )
                    ins.sync_info = mybir.SyncInfo(
                        on_wait=list(si.on_wait), on_update=[]
                    )
                    insts.insert(i + 1, ev)
                    i += 1
                i += 1


def _get_built():
    global _BUILT
    if _BUILT is None:
        _BUILT = _build()
        _split_multi_waits(_BUILT)
    return _BUILT


def _host_inputs(X, wk, wq, wv, Wp, bp, W1, b1, W2, b2, w_lm, b_lm):
    c = wk * wq  # [L,H]
    wvp = wv * Wp[:, :, 0]  # [L,H]
    # [j,i] = 1 if j<i; 2^-33 scale keeps ln(den') in the Ln LUT window
    trin = np.triu(np.ones((T, T), np.float32), 1) * 2.0**-33
    trid = trin.copy()
    trid[0, 0] = 2.0**-33  # den row0 = E[0,:] keeps den>0; num row0 stays 0

    XT = np.ascontiguousarray(X.T.astype(np.float32))  # [T, B]

    # smalls (identical across cores): [T, SC]
    sm_common = np.zeros((1, SC), np.float32)
    for l in range(L):
        base = CB0 + l * 16
        sm_common[0, base : base + 8] = c[l]
        sm_common[0, base + 8 : base + 16] = wvp[l]
    for l in range(L):
        fb = FF0 + l * FBLK
        lm_s = float(w_lm[0]) if l == L - 1 else 1.0
        lm_b = float(b_lm[0]) if l == L - 1 else 0.0
        sm_common[0, fb : fb + 4] = W1[l, 0, :]
        sm_common[0, fb + 4 : fb + 8] = W1[l, 0, :] * bp[l, 0] + b1[l]
        sm_common[0, fb + 8 : fb + 12] = W2[l, :, 0] * lm_s
        sm_common[0, fb + 12] = lm_s
        sm_common[0, fb + 13] = (bp[l, 0] + b2[l, 0]) * lm_s + lm_b
    sm = np.ascontiguousarray(np.broadcast_to(sm_common, (T, SC)), np.float32)

    in_maps = []
    for core in range(NCORES):
        smc = sm.copy()
        smc[:, XT0 : XT0 + BC] = XT[:, core * BC : (core + 1) * BC]
        in_maps.append(
            {
                "trid": trid,
                "trin": trin,
                "smalls": np.ascontiguousarray(smc),
            }
        )
    return in_maps


def kernel(X, wk, wq, wv, Wp, bp, W1, b1, W2, b2, w_lm, b_lm):
    global LAST_RESULT
    args = [
        np.asarray(a, np.float32)
        for a in (X, wk, wq, wv, Wp, bp, W1, b1, W2, b2, w_lm, b_lm)
    ]
    nc = _get_built()
    in_maps = _host_inputs(*args)
    res = run_bass_kernel_spmd(nc, in_maps, core_ids=list(range(NCORES)))
    LAST_RESULT = res

    out = np.empty((B, T), np.float32)
    for core in range(NCORES):
        out[core * BC : (core + 1) * BC, :] = res.results[core]["out_t"].T
    return out


# revision 26
# speedup vs baseline: 1.0874x; 1.0019x over previous
"""Trainium2 Bass kernel for nn_CaT_13941463842986 (sparse_attention).

Math (head_size==1 collapses attention to a prefix softmax over T):
  qk[b,h,j]   = c[l,h] * x[b,j]^2            with c = wk*wq
  head_out    = (excl-prefix-sum of E*v) / (excl-prefix-sum of E),
  E = exp(qk), v = x*wv.  Exclusive prefix sums over T=128 are matmuls
against strict-upper-triangular (in [j,i] indexing) ones matrices on the
tensor engine.  |qk| <= ~49 for this data, so exp() needs no max-shift.

Sharding: pure data parallel over batch B=512 -> 64 rows per core x 8 cores.
On-chip layout is T-major: tiles are [T=128 partitions, (b,h) free],
free index = b*8 + h (h innermost).

Key techniques:
 - per-head broadcasts (x, x^2, per-head consts) are stride-0 access
   patterns directly on DVE compute ops -- no broadcast DMAs at all
 - 1/den comes from the ACT LUTs: r = exp(-ln(den)); both tri matrices
   carry a 2^-33 scale so ln's input stays inside the LUT's ~2^+-64
   window (num and den scale together, so num'*r' == num/den)
 - the head sum is one strided X-axis tensor_reduce over [T, 64, 8]
 - tri_den[0,0]=1 keeps den>0 on row 0; tri_num keeps the 0 so no
   row-0 fixup is needed
 - per-chunk tiles + tuned emission order keep every consumer's
   engine-counter wait on its true producer; ev is computed as
   (ee*x)*wvp so no DVE op is ready before the critical qk, which
   the greedy readiness scheduler would otherwise delay
 - everything elementwise lives on DVE/ACT: GpSimd shares SBUF ports
   with DVE and measurably halves both engines' throughput when run
   concurrently
 - all input-derived scalars ride tiles/APs, so the built program is
   input-independent
"""

import numpy as np

import concourse.bass as bass
import concourse.mybir as mybir
from concourse import tile
from concourse.alu_op_type import AluOpType
from concourse.bass_utils import run_bass_kernel_spmd

B, T, H, L = 512, 128, 8, 3
NCORES = 8
BC = B // NCORES  # 64 batch rows per core
W = H * BC  # 512 free width of the (b,h) tiles
HW2 = W // 2
F32 = mybir.dt.float32
F32R = mybir.dt.float32r
AF = mybir.ActivationFunctionType

# ffc const-tile column layout (per layer l at FF0 + l*FBLK):
#   0:4   w1   (W1[l,0,k])
#   4:8   b1'  (W1[l,0,k]*bp[l] + b1[l,k])
#   8:12  w2'  (W2[l,k,0], *w_lm for l==2)
#   12    ybb scale   (1.0, w_lm for l==2)
#   13    ybb bias    (bp+b2, *w_lm + b_lm for l==2)
FBLK = 16
# smalls tensor: [T, SC] = xt(64) | cb_l(8) wb_l(8) x3 | ff(48)
XT0, CB0 = 0, 64
FF0 = CB0 + L * 16
SC = FF0 + L * FBLK

LAST_RESULT = None
_BUILT = None

CSL = [slice(0, HW2), slice(HW2, W)]  # wide column chunks (b 0:32 | 32:64)


def _bcast_bh(xcol, bsl=None):
    """[T,64] tile -> [T,nb,8] stride-0 view (replicate along h)."""
    v = xcol if bsl is None else xcol[:, bsl]
    nb = v.shape[1]
    return v.unsqueeze(2).broadcast_to([T, nb, 8])


def _bcast_h(hrow, nb):
    """[T,8] tile -> [T,nb,8] stride-0 view (replicate along b)."""
    return hrow[:, :].unsqueeze(1).broadcast_to([T, nb, 8])


def _w3(tile_, csl):
    """[T,W] tile chunk -> [T,nb,8] view."""
    return tile_[:, csl].rearrange("p (b h) -> p b h", h=H)


def _build():
    nc = bass.Bass("TRN2", target_bir_lowering=False, debug=False)

    trid_d = nc.dram_tensor("trid", [T, T], F32R, kind="ExternalInput")
    trin_d = nc.dram_tensor("trin", [T, T], F32R, kind="ExternalInput")
    sm_d = nc.dram_tensor("smalls", [T, SC], F32, kind="ExternalInput")
    out_d = nc.dram_tensor("out_t", [T, BC], F32, kind="ExternalOutput")

    with tile.TileContext(nc) as tc:
        with tc.tile_pool(name="const", bufs=1) as cp, tc.tile_pool(
            name="work", bufs=3
        ) as wp, tc.tile_pool(name="psum", bufs=2, space="PSUM") as pp:
            trid = cp.tile([T, T], F32R, tag="trid")
            trin = cp.tile([T, T], F32R, tag="trin")
            sm = cp.tile([T, SC], F32, tag="sm")

            # trigger the ACT table load right away (input values are
            # irrelevant -- this is only a warmup for the LUT load)
            scratch = cp.tile([T, 1], F32, tag="scratch")
            nc.scalar.activation(
                out=scratch[:, :], in_=scratch[:, :], func=AF.Exp
            )

            # loads ride both HW-DGE queues, ordered by first use:
            # sm (x + consts) -> trid (den matmul) -> trin
            SH = SC // 2
            nc.sync.dma_start(out=sm[:, :SH], in_=sm_d[:, :SH])
            nc.scalar.dma_start(out=trid[:, T // 2 :], in_=trid_d[:, T // 2 :])
            nc.sync.dma_start(out=trid[:, : T // 2], in_=trid_d[:, : T // 2])
            nc.scalar.dma_start(out=sm[:, SH:], in_=sm_d[:, SH:])
            nc.sync.dma_start(out=trin[:, : T // 2], in_=trin_d[:, : T // 2])
            nc.scalar.dma_start(out=trin[:, T // 2 :], in_=trin_d[:, T // 2 :])

            xcur = sm[:, XT0 : XT0 + BC]
            for l in range(L):
                fb = FF0 + l * FBLK
                cb = sm[:, CB0 + l * 16 : CB0 + l * 16 + 8]
                wb = sm[:, CB0 + l * 16 + 8 : CB0 + l * 16 + 16]
                u = wp.tile([T, BC], F32, tag="u")
                nc.vector.tensor_tensor(
                    out=u[:, :], in0=xcur[:, :], in1=xcur[:, :],
                    op=AluOpType.mult,
                )
                # qk in two chunks so exp_a can start after qk_a; no
                # other DVE op is ready before qk (ev depends on ee),
                # so the greedy scheduler cannot delay the chain
                qk = wp.tile([T, W], F32, tag="qk")
                for ci in (0, 1):
                    bsl = slice(ci * 32, (ci + 1) * 32)
                    nc.vector.tensor_tensor(
                        out=_w3(qk, CSL[ci]),
                        in0=_bcast_bh(u, bsl),
                        in1=_bcast_h(cb, 32),
                        op=AluOpType.mult,
                    )

                # per-chunk tiles: dependency tracking is tile-granular,
                # so separate tiles let each consumer start as soon as its
                # own chunk's producer is done
                ee = [wp.tile([T, HW2], F32R, tag=f"ee{c}", name=f"ee{c}") for c in (0, 1)]
                ev = [wp.tile([T, HW2], F32R, tag=f"ev{c}", name=f"ev{c}") for c in (0, 1)]
                den = [pp.tile([T, HW2], F32, tag=f"den{c}", name=f"den{c}") for c in (0, 1)]
                num = [pp.tile([T, HW2], F32, tag=f"num{c}", name=f"num{c}") for c in (0, 1)]
                for ci in (0, 1):
                    csl = CSL[ci]
                    bsl = slice(ci * 32, (ci + 1) * 32)
                    nc.scalar.activation(
                        out=ee[ci][:, :], in_=qk[:, csl], func=AF.Exp
                    )
                    # ev = (ee * x_bcast) * wvp_bcast; making both
                    # multiplies depend on ee keeps the DVE ready-queue
                    # empty until qk is done
                    e2 = wp.tile([T, HW2], F32, tag=f"e2{ci}", name=f"e2{ci}")
                    nc.vector.tensor_tensor(
                        out=_w3(e2, slice(0, HW2)),
                        in0=ee[ci][:, :].rearrange("p (b h) -> p b h", h=H),
                        in1=_bcast_bh(xcur, bsl),
                        op=AluOpType.mult,
                    )
                    nc.vector.tensor_tensor(
                        out=_w3(ev[ci], slice(0, HW2)),
                        in0=_w3(e2, slice(0, HW2)),
                        in1=_bcast_h(wb, 32),
                        op=AluOpType.mult,
                    )
                # PE order: both den (tri_den loaded once), then both num
                for ci in (0, 1):
                    nc.tensor.matmul(
                        den[ci][:, :], trid[:, :], ee[ci][:, :],
                        start=True, stop=True,
                    )
                for ci in (0, 1):
                    nc.tensor.matmul(
                        num[ci][:, :], trin[:, :], ev[ci][:, :],
                        start=True, stop=True,
                    )

                # r = 1/den = exp(-ln(den)); ho = num * r, chunk-pipelined
                # against ACT
                ho = wp.tile([T, W], F32, tag="ho")
                for ci in (0, 1):
                    csl = CSL[ci]
                    ld = wp.tile([T, HW2], F32, tag=f"ld{ci}", name=f"ld{ci}")
                    r = wp.tile([T, HW2], F32, tag=f"r{ci}", name=f"r{ci}")
                    nc.scalar.activation(
                        out=ld[:, :], in_=den[ci][:, :], func=AF.Ln
                    )
                    nc.scalar.activation(
                        out=r[:, :], in_=ld[:, :], func=AF.Exp, scale=-1.0
                    )
                    nc.vector.tensor_tensor(
                        out=ho[:, csl], in0=num[ci][:, :], in1=r[:, :],
                        op=AluOpType.mult,
                    )
                y0 = wp.tile([T, BC], F32, tag="y0")
                nc.vector.tensor_reduce(
                    out=y0[:, :],
                    in_=ho[:, :].rearrange("p (b h) -> p b h", h=H),
                    axis=mybir.AxisListType.X,
                    op=AluOpType.add,
                )

                # FF: xn = ybb + sum_k w2'_k * relu(w1_k*y0 + b1'_k)
                ybb = wp.tile([T, BC], F32, tag="ybb")
                nc.vector.tensor_scalar(
                    out=ybb[:, :], in0=y0[:, :],
                    scalar1=sm[:, fb + 12 : fb + 13],
                    scalar2=sm[:, fb + 13 : fb + 14],
                    op0=AluOpType.mult,
                    op1=AluOpType.add,
                )
                rk = wp.tile([T, 4 * BC], F32, tag="rk")
                for k in range(4):
                    nc.scalar.activation(
                        out=rk[:, k * BC : (k + 1) * BC], in_=y0[:, :],
                        func=AF.Relu,
                        scale=sm[:, fb + k : fb + k + 1],
                        bias=sm[:, fb + 4 + k : fb + 5 + k],
                    )
                q = ybb
                for k in range(4):
                    qn = wp.tile([T, BC], F32, tag=f"q{k}", name=f"q{k}")
                    nc.vector.scalar_tensor_tensor(
                        out=qn[:, :],
                        in0=rk[:, k * BC : (k + 1) * BC],
                        scalar=sm[:, fb + 8 + k : fb + 9 + k],
                        in1=q[:, :],
                        op0=AluOpType.mult,
                        op1=AluOpType.add,
                    )
                    q = qn
                xcur = q

            nc.sync.dma_start(out=out_d[:, :], in_=xcur[:, :])

    return nc


def _split_multi_waits(nc):
    """This container's walrus accepts only one embedded sem wait per
    instruction; hoist extra waits onto same-engine EventSemaphore ops.
    Custom-DVE ISA ops can't carry any embedded sync at all."""
    nid = 0
    for fn in nc.m.functions:
        for blk in fn.blocks:
            insts = blk.instructions
            i = 0
            while i < len(insts):
                ins = insts[i]
                si = getattr(ins, "sync_info", None)
                is_custom = isinstance(ins, mybir.InstCustomDveAnt)
                is_raw_isa = isinstance(ins, mybir.InstISA) and not is_custom
                keep = 0 if is_custom else 1
                if si is not None and len(si.on_wait) > keep and not is_raw_isa:
                    waits = list(si.on_wait)
                    split, kept = (
                        (waits, []) if keep == 0 else (waits[:-1], [waits[-1]])
                    )
                    for w in split:
                        ev = mybir.InstEventSemaphore(
                            name=f"WSPLIT-{nid}", ins=[], outs=[]
                        )
                        nid += 1
                        ev.engine = ins.engine
                        ev.sync_info = mybir.SyncInfo(on_wait=[w], on_update=[])
                        insts.insert(i, ev)
                        i += 1
                    ins.sync_info = mybir.SyncInfo(
                        on_wait=kept, on_update=list(si.on_update)
                    )
                    si = ins.sync_info
                if is_custom and si is not None and len(si.on_update) > 0:
                    ev = mybir.InstEventSemaphore(
                        name=f"WSPLIT-{nid}", ins=[], outs=[]
                    )
                    nid += 1
                    ev.engine = ins.engine
                    ev.sync_info = mybir.SyncInfo(
                        on_wait=[], on_update=list(si.on_update)
                    )
                    ins.sync_info = mybir.SyncInfo(
                        on_wait=list(si.on_wait), on_update=[]
                    )
                    insts.insert(i + 1, ev)
                    i += 1
                i += 1


def _get_built():
    global _BUILT
    if _BUILT is None:
        _BUILT = _build()
        _split_multi_waits(_BUILT)
    return _BUILT


def _host_inputs(X, wk, wq, wv, Wp, bp, W1, b1, W2, b2, w_lm, b_lm):
    c = wk * wq  # [L,H]
    wvp = wv * Wp[:, :, 0]  # [L,H]
    # [j,i] = 1 if j<i; 2^-33 scale keeps ln(den') in the Ln LUT window
    trin = np.triu(np.ones((T, T), np.float32), 1) * 2.0**-33
    trid = trin.copy()
    trid[0, 0] = 2.0**-33  # den row0 = E[0,:] keeps den>0; num row0 stays 0

    XT = np.ascontiguousarray(X.T.astype(np.float32))  # [T, B]

    # smalls (identical across cores): [T, SC]
    sm_common = np.zeros((1, SC), np.float32)
    for l in range(L):
        base = CB0 + l * 16
        sm_common[0, base : base + 8] = c[l]
        sm_common[0, base + 8 : base + 16] = wvp[l]
    for l in range(L):
        fb = FF0 + l * FBLK
        lm_s = float(w_lm[0]) if l == L - 1 else 1.0
        lm_b = float(b_lm[0]) if l == L - 1 else 0.0
        sm_common[0, fb : fb + 4] = W1[l, 0, :]
        sm_common[0, fb + 4 : fb + 8] = W1[l, 0, :] * bp[l, 0] + b1[l]
        sm_common[0, fb + 8 : fb + 12] = W2[l, :, 0] * lm_s
        sm_common[0, fb + 12] = lm_s
        sm_common[0, fb + 13] = (bp[l, 0] + b2[l, 0]) * lm_s + lm_b
    sm = np.ascontiguousarray(np.broadcast_to(sm_common, (T, SC)), np.float32)

    in_maps = []
    for core in range(NCORES):
        smc = sm.copy()
        smc[:, XT0 : XT0 + BC] = XT[:, core * BC : (core + 1) * BC]
        in_maps.append(
            {
                "trid": trid,
                "trin": trin,
                "smalls": np.ascontiguousarray(smc),
            }
        )
    return in_maps


def kernel(X, wk, wq, wv, Wp, bp, W1, b1, W2, b2, w_lm, b_lm):
    global LAST_RESULT
    args = [
        np.asarray(a, np.float32)
        for a in (X, wk, wq, wv, Wp, bp, W1, b1, W2, b2, w_lm, b_lm)
    ]
    nc = _get_built()
    in_maps = _host_inputs(*args)
    res = run_bass_kernel_spmd(nc, in_maps, core_ids=list(range(NCORES)))
    LAST_RESULT = res

    out = np.empty((B, T), np.float32)
    for core in range(NCORES):
        out[core * BC : (core + 1) * BC, :] = res.results[core]["out_t"].T
    return out
